# revision 19
# baseline (speedup 1.0000x reference)
"""Trainium2 Bass kernel for the LocalAggregator nn.Module.

Reference computation:
    power[p,g]  = -0.5 * d^T Prec_g d          (d = pts[p] - means3D[g])
    within[p,g] = all(|voxel(pts[p]) - voxel(means3D[g])| <= radii[g])
    logits      = where(within & power<=0, exp(power), 0) @ opacities

Device algorithm (everything O(P*G) runs on the NeuronCores):
  * power is a quadratic polynomial in the point coordinates -> a matmul of
    per-point feature rows against per-gaussian coefficient columns.  Both
    sides are triple-split into bf16 (w=w1+w2+w3 exactly); the six combos
    w1f1,w1f2,w2f1,w2f2,w1f3,w3f1 reproduce fp32-level accuracy (dropped
    terms are O(2^-27 * |w||f|)) at bf16 matmul speed (1 cycle/column).
    (float32r would be as fast, but its walrus lowering poisons any
    subsequent matmul issued with start_tensor_calc=False.)
  * the voxel box test is EXACT via a one-hot matmul in fp8 DoubleRow mode
    (0.5 cycle/column): one-hot voxel rows (value 224) x {0,1} interval
    indicator columns add 224*(#axes within); the constant row carries
    -3*224 so non-within pairs land below exp's fp32 underflow (-104).
  * ScalarE evaluates exp from PSUM into bf16, then the opacity matmul is
    TRANSPOSED: stationary = weights [128g x 128p], moving = opacities
    [128g x 18] -> 18-cycle matmuls accumulating logits [p,18] slices in a
    single PSUM bank.
Sharding: 8 x-columns of 2048 points (one per core); per core 6 y-shards of
[384,384,384,384,256,256] points, each with <=128 exactly-culled gaussians
(occupancy test, not bbox).  Coordinates re-centered per shard.
"""

import numpy as np
import ml_dtypes

import concourse.bass as bass
import concourse.mybir as mybir
import concourse.tile as tile
import concourse.bass2jax as _bass2jax
import concourse.bass_utils as _bass_utils
from concourse.bass_utils import run_bass_kernel_spmd

import json as _json


def _split_waits(bir_json):
    """Walrus in this toolchain rejects instructions carrying more than one
    sync wait ("Too many sync wait commands").  Split every multi-wait
    instruction into a chain of single-wait NoOps on the same engine (program
    order on the engine's sequencer preserves the wait-before-op semantics)."""
    if isinstance(bir_json, (bytes, bytearray)):
        m = _json.loads(bir_json.decode())
    else:
        m = _json.loads(bir_json)
    cnt = 0
    for f in m["functions"]:
        for bb in f["blocks"]:
            new_insts = []
            for inst in bb["instructions"]:
                si = inst.get("sync_info")
                waits = (si or {}).get("on_wait") or []
                if len(waits) > 1:
                    eng = inst.get("engine")
                    for w in waits[:-1]:
                        cnt += 1
                        nop = {
                            "debug": 16,
                            "ins": [],
                            "name": f"I-nopw-{cnt}",
                            "opcode": "NoOp",
                            "outs": [],
                            "sync_info": {"on_update": [], "on_wait": [w]},
                        }
                        if eng is not None:
                            nop["engine"] = eng
                        new_insts.append(nop)
                    si["on_wait"] = [waits[-1]]
                new_insts.append(inst)
            bb["instructions"] = new_insts
    return _json.dumps(m).encode()


_orig_compile_bir_kernel = _bass_utils.compile_bir_kernel.__wrapped__ if hasattr(
    _bass_utils.compile_bir_kernel, "__wrapped__") else _bass_utils.compile_bir_kernel


def _patched_compile_bir_kernel(bir_json, tmpdir, neff_name="file.neff"):
    return _orig_compile_bir_kernel(_split_waits(bir_json), tmpdir, neff_name)


_bass2jax.compile_bir_kernel = _patched_compile_bir_kernel
_bass_utils.compile_bir_kernel = _patched_compile_bir_kernel

GRID = np.float32(0.5)
SCALE_MULT = np.float32(3.0)
MPEN = 224.0  # penalty unit; exact in float8_e4m3, 3*MPEN >> 104 (exp underflow)
N_CORES = 8
FP8_NP = ml_dtypes.float8_e4m3
C = 18
P_CORE = 2048
PATTERN = (384, 384, 384, 384, 256, 256)  # per-core y-shard point counts

_nc_cache = {}


def _build_bass(R, S2, pblocks, gts, n_c):
    """R: bf16 feature rows; S2: fp8 one-hot rows per k-tile; pblocks: per-shard
    point counts; gts: per-shard gaussian tile counts; n_c: C."""
    f32 = mybir.dt.float32
    fp8 = mybir.dt.float8e4
    bf16 = mybir.dt.bfloat16
    DR = mybir.MatmulPerfMode.DoubleRow
    Exp = mybir.ActivationFunctionType.Exp

    NS = len(pblocks)
    NG = sum(gts)
    P = sum(pblocks)
    NB = P // 128
    OBB = NG * n_c * 2 + 8  # opacity bytes + f32 {0.0, 1.0} consts
    SR = max(R, S2)         # bundle partition rows

    # byte-bundle layout (per bundle partition row): coefficient sections
    # first, then per-shard feature sections (bf16 features + fp8 one-hot).
    fqw_off = 0                       # [R, NG*128] bf16
    ohw_off = NG * 256                # [S2, NG, 2, 128] fp8
    sh_off = []
    acc = 2 * NG * 256
    for ps in pblocks:
        sh_off.append(acc)            # fq-s at acc (2*ps bytes), oh-s follows
        acc += 4 * ps
    TOT = acc
    # chunk boundaries: [W + shard0 | shards 1-2 | shards 3..]
    cut1 = sh_off[1]
    cut2 = sh_off[3]

    nc = bass.Bass()
    bun_d = nc.dram_tensor("bun", [SR, TOT], mybir.dt.uint8, kind="ExternalInput")
    ob_d = nc.dram_tensor("ob", [128, OBB], mybir.dt.uint8, kind="ExternalInput")
    out_d = nc.dram_tensor("out", [128, NB, n_c], f32, kind="ExternalOutput")

    with tile.TileContext(nc) as tc:
        with (
            tc.tile_pool(name="singles", bufs=1) as singles,
            tc.tile_pool(name="wpool", bufs=3) as wpool,
            tc.tile_pool(name="pp", bufs=3, space="PSUM") as pp,
            tc.tile_pool(name="pl", bufs=1, space="PSUM") as pl,
        ):
            bun = singles.tile([SR, TOT], mybir.dt.uint8)
            ob_sb = singles.tile([128, OBB], mybir.dt.uint8)
            osb = singles.tile([128, NB * n_c], f32)

            nc.sync.dma_start(out=bun[:, :cut1], in_=bun_d[:, :cut1])
            nc.sync.dma_start(out=bun[:, cut1:cut2], in_=bun_d[:, cut1:cut2])
            nc.sync.dma_start(out=bun[:, cut2:], in_=bun_d[:, cut2:])
            nc.gpsimd.dma_start(out=ob_sb[:], in_=ob_d[:])

            opa = ob_sb[:, 0:OBB - 8].bitcast(bf16)  # [128, NG*C]
            zero_ap = ob_sb[:, OBB - 8:OBB - 4].bitcast(f32)
            one_ap = ob_sb[:, OBB - 4:OBB].bitcast(f32)

            def fqw_v(gi):
                return bun[0:R, fqw_off + 256 * gi:fqw_off + 256 * (gi + 1)
                           ].bitcast(bf16)

            def ohw_v(gi):
                return bun[0:S2, ohw_off + 256 * gi:ohw_off + 256 * (gi + 1)
                           ].bitcast(fp8).rearrange("p (two c) -> p two c", two=2)

            def fqf_v(s, ps):
                return bun[0:R, sh_off[s]:sh_off[s] + 2 * ps].bitcast(bf16)

            def ohf_v(s, ps):
                return bun[0:S2, sh_off[s] + 2 * ps:sh_off[s] + 4 * ps
                           ].bitcast(fp8).rearrange("p (two c) -> p two c", two=2)

            # per-shard bookkeeping
            goff = [sum(gts[:s]) for s in range(NS)]
            poff = [sum(pblocks[:s]) for s in range(NS)]
            boff = [poff[s] // 128 for s in range(NS)]

            # logits accumulate in two PSUM tiles so the big head DMA can
            # leave while the last shard is still computing
            nbA = boff[NS - 3]
            pslA = pl.tile([128, nbA * n_c], f32, name="pslA")
            pslB = pl.tile([128, (NB - nbA) * n_c], f32, name="pslB")
            finA = sum(gts[s] * (pblocks[s] // 128) for s in range(NS - 3))
            finB = sum(gts[s] * (pblocks[s] // 128) for s in range(NS - 3, NS))
            fin_i = 0

            psp_tiles = [None] * NS
            wt_tiles = [None] * NS

            pmax = max(pblocks)

            def emit_power(s):
                ps = pblocks[s]
                tiles = []
                for t in range(gts[s]):
                    gi = goff[s] + t
                    psp = pp.tile([128, pmax], f32, name="psp")[:, :ps]
                    nc.tensor.matmul(
                        psp[:], fqw_v(gi), fqf_v(s, ps),
                        start=True, stop=False,
                    )
                    nc.tensor.matmul(
                        psp[:], ohw_v(gi), ohf_v(s, ps),
                        start=False, stop=True, perf_mode=DR,
                    )
                    tiles.append(psp)
                psp_tiles[s] = tiles

            def emit_exp(s):
                ps = pblocks[s]
                tiles = []
                for t in range(gts[s]):
                    wt = wpool.tile([128, pmax], bf16, name="wt")[:, :ps]
                    nc.scalar.activation(
                        out=wt[:], in_=psp_tiles[s][t][:], func=Exp,
                        bias=zero_ap, scale=one_ap, alpha=zero_ap,
                    )
                    tiles.append(wt)
                wt_tiles[s] = tiles

            def emit_final(s):
                nonlocal fin_i
                ps = pblocks[s]
                last = s >= NS - 3
                psl, nfin, base = (
                    (pslB, finB, nbA) if last else (pslA, finA, 0))
                if s == NS - 3:
                    fin_i = 0
                for t in range(gts[s]):
                    gi = goff[s] + t
                    wt = wt_tiles[s][t]
                    for b in range(ps // 128):
                        cs = (boff[s] + b - base) * n_c
                        nc.tensor.matmul(
                            psl[:, cs:cs + n_c],
                            wt[:, 128 * b:128 * (b + 1)],
                            opa[:, gi * n_c:(gi + 1) * n_c],
                            start=(fin_i == 0), stop=(fin_i == nfin - 1),
                        )
                        fin_i += 1

            def emit_out(blk0, blk1, eng):
                """Copy psl block range to SBUF and DMA to DRAM on `eng`'s
                HWDGE queue (spreads SEQ + HWDGE issue cost across engines)."""
                cs, ce = blk0 * n_c, blk1 * n_c
                psl = pslB if blk0 >= nbA else pslA
                base = nbA * n_c if blk0 >= nbA else 0
                nc.vector.tensor_copy(
                    out=osb[:, cs:ce], in_=psl[:, cs - base:ce - base])
                eng.dma_start(out=out_d[:, blk0:blk1, :], in_=osb[:, cs:ce])

            # software pipeline: power mms run ahead; exp as soon as each
            # shard's psum closes; finals trail one shard behind.  Output
            # leaves in two DMAs: shards 0..NS-2 (issued while the last
            # shard computes, from the DVE queue) and the last shard alone
            # (short tail transfer, from the idle SP queue).
            emit_power(0)
            emit_exp(0)
            for s in range(1, NS):
                emit_power(s)
                emit_exp(s)
                emit_final(s - 1)
            emit_out(0, nbA, nc.sync)
            emit_final(NS - 1)
            emit_out(nbA, NB, nc.scalar)
    return nc


BF16 = ml_dtypes.bfloat16
# combo i pairs w-part WCOMBO[i] with f-part FCOMBO[i]; the six combos cover
# every product pair down to O(2^-27).
WCOMBO = (0, 0, 1, 1, 0, 2)
FCOMBO = (0, 1, 0, 1, 2, 0)


def _tsplit(x):
    """Exact bf16 triple split of a float64 array: x ~= x1+x2+x3."""
    x = np.asarray(x, np.float64)
    x1 = x.astype(BF16)
    r1 = x - x1.astype(np.float64)
    x2 = r1.astype(BF16)
    x3 = (r1 - x2.astype(np.float64)).astype(BF16)
    return x1, x2, x3


def _prepare(inputs):
    """Host-side O(P+G) prep: sharding, culling, feature/coefficient packing."""
    pts = np.ascontiguousarray(np.asarray(inputs["pts"], dtype=np.float32))
    means3D = np.ascontiguousarray(np.asarray(inputs["means3D"], dtype=np.float32))
    opac = np.asarray(inputs["opacities"], dtype=np.float32)
    scales = np.asarray(inputs["scales"], dtype=np.float32)
    cov3D = np.asarray(inputs["cov3D"], dtype=np.float32)
    pc_min = np.asarray(inputs["pc_min"], dtype=np.float32)

    P, G = pts.shape[0], means3D.shape[0]
    n_c = opac.shape[1]
    assert P == N_CORES * P_CORE

    # integer voxel quantities, identical fp32 arithmetic to the reference
    pts_int = np.floor((pts - pc_min[None, :]) / GRID).astype(np.int32)
    means_int = np.floor((means3D - pc_min[None, :]) / GRID).astype(np.int32)
    radii = np.ceil(scales.max(-1) * SCALE_MULT / GRID).astype(np.int32)
    cov6 = cov3D.reshape(G, 9)[:, [0, 4, 8, 1, 5, 2]].astype(np.float64)
    has_cross = bool(np.abs(cov6[:, 3:]).max() > 0.0)

    a_, b_, c_ = cov6[:, 0], cov6[:, 1], cov6[:, 2]
    pxy, pyz, pxz = cov6[:, 3], cov6[:, 4], cov6[:, 5]

    # spatial sharding: 8 x-columns (by sorted order) -> cores; 6 y-shards each
    order = np.argsort(pts_int[:, 0], kind="stable")
    cores = []
    for xs in range(4):
        chunk = order[xs * 4096:(xs + 1) * 4096]
        sub = chunk[np.argsort(pts_int[chunk, 1], kind="stable")]
        cores.append(sub[:P_CORE])
        cores.append(sub[P_CORE:])

    NS = len(PATTERN)
    poff = [sum(PATTERN[:s]) for s in range(NS)]

    # exact culling + per-shard metadata
    shard_info = []  # [core][shard] -> (idx, gsel, lo, hi)
    gts = [1] * NS
    smax = 1
    for ci in range(N_CORES):
        rows = []
        for s in range(NS):
            idx = cores[ci][poff[s]:poff[s] + PATTERN[s]]
            pi = pts_int[idx]
            lo, hi = pi.min(0), pi.max(0)
            cand = np.where(
                (means_int >= lo - radii[:, None]).all(1)
                & (means_int <= hi + radii[:, None]).all(1)
            )[0]
            keep = [g for g in cand
                    if (np.abs(pi - means_int[g]) <= radii[g]).all(1).any()]
            gsel = np.asarray(keep, dtype=np.int64)
            rows.append((idx, gsel, lo, hi))
            gts[s] = max(gts[s], (max(len(gsel), 1) + 127) // 128)
            smax = max(smax, int((hi - lo + 1).sum()))
        shard_info.append(rows)

    gts = tuple(gts)
    S2 = (smax + 1) // 2
    NG = sum(gts)
    goff = [sum(gts[:s]) for s in range(NS)]
    CFW = 128 * NG
    CF = CFW + P_CORE
    COW = 128 * NG
    CO = COW + P_CORE
    OBB = NG * n_c * 2 + 8

    base_rows = 10 if has_cross else 7  # quad + linear + const
    R = 6 * base_rows

    in_maps = []
    for ci in range(N_CORES):
        FQ = np.zeros((R, CF), BF16)
        OH = np.zeros((S2, 2, CO), FP8_NP)
        OPA = np.zeros((128, NG, n_c), ml_dtypes.bfloat16)

        for s in range(NS):
            idx, gsel, lo, hi = shard_info[ci][s]
            ps = PATTERN[s]
            gl = len(gsel)
            cen = (lo + hi + 1).astype(np.float64) * (0.5 * float(GRID))
            p64 = pts[idx].astype(np.float64) - cen
            m64 = means3D[gsel].astype(np.float64) - cen
            x, y, z = p64[:, 0], p64[:, 1], p64[:, 2]
            mx, my, mz = m64[:, 0], m64[:, 1], m64[:, 2]
            ag, bg, cg = a_[gsel], b_[gsel], c_[gsel]

            if has_cross:
                pxyg, pyzg, pxzg = pxy[gsel], pyz[gsel], pxz[gsel]
                feats = [x * x, y * y, z * z, x * y, y * z, x * z,
                         x, y, z, np.ones_like(x)]
                Amx = ag * mx + pxyg * my + pxzg * mz
                Amy = pxyg * mx + bg * my + pyzg * mz
                Amz = pxzg * mx + pyzg * my + cg * mz
                mAm = mx * Amx + my * Amy + mz * Amz
                coefs = [-0.5 * ag, -0.5 * bg, -0.5 * cg, -pxyg, -pyzg, -pxzg,
                         Amx, Amy, Amz, -0.5 * mAm - 3.0 * MPEN]
            else:
                feats = [x * x, y * y, z * z, x, y, z, np.ones_like(x)]
                mAm = ag * mx * mx + bg * my * my + cg * mz * mz
                coefs = [-0.5 * ag, -0.5 * bg, -0.5 * cg,
                         ag * mx, bg * my, cg * mz, -0.5 * mAm - 3.0 * MPEN]

            fcol = CFW + poff[s]
            # padded gaussian columns: all-zero coefs except const -> exp(-672)=0
            gcol = 128 * goff[s]
            gpad = 128 * gts[s]
            for r in range(base_rows):
                fp = _tsplit(feats[r])
                wp = _tsplit(coefs[r])
                for i in range(6):
                    FQ[i * base_rows + r, fcol:fcol + ps] = fp[FCOMBO[i]]
                    FQ[i * base_rows + r, gcol:gcol + gl] = wp[WCOMBO[i]]
            cr = base_rows - 1  # const row: fill padded gaussian columns
            for i in range(6):
                if WCOMBO[i] == 0:
                    FQ[i * base_rows + cr, gcol + gl:gcol + gpad] = BF16(-3.0 * MPEN)
            # one-hot axes: order z, x, y
            span = (hi - lo + 1).astype(np.int64)
            axes = [2, 0, 1]
            offs = np.zeros(3, np.int64)
            acc = 0
            for ax in axes:
                offs[ax] = acc
                acc += int(span[ax])
            tcol = np.arange(ps)
            for ax in axes:
                flat = offs[ax] + (pts_int[idx, ax] - lo[ax])
                OH[flat % S2, flat // S2, fcol + tcol] = FP8_NP(MPEN)
            for ax in axes:
                sa = int(span[ax])
                blo = np.maximum(means_int[gsel, ax] - radii[gsel] - lo[ax], 0)
                bhi = np.minimum(means_int[gsel, ax] + radii[gsel] - lo[ax], sa - 1)
                k = np.arange(sa)[:, None]
                box = ((k >= blo[None, :]) & (k <= bhi[None, :]))
                flat = offs[ax] + np.arange(sa)
                OH[flat % S2, flat // S2, gcol:gcol + gl] = np.where(
                    box, FP8_NP(1.0), FP8_NP(0.0))
            OPA[:gl, goff[s], :] = opac[gsel].astype(ml_dtypes.bfloat16)
            if gts[s] > 1:
                # split gsel across tiles (gl>128)
                OPA[:, goff[s]:goff[s] + gts[s], :] = 0
                for t in range(gts[s]):
                    seg = gsel[128 * t:128 * (t + 1)]
                    OPA[:len(seg), goff[s] + t, :] = opac[seg].astype(
                        ml_dtypes.bfloat16)
                # redo coefficient columns per tile
                # (handled above only for t=0; rebuild full block)
                for r in range(R):
                    FQ[r, gcol:gcol + gpad] = 0
                OH[:, :, gcol:gcol + gpad] = FP8_NP(0.0)
                for t in range(gts[s]):
                    seg = np.arange(128 * t, min(128 * (t + 1), gl))
                    gc2 = gcol + 128 * t
                    n2 = len(seg)
                    for r in range(base_rows):
                        wp = _tsplit(coefs[r][seg])
                        for i in range(6):
                            FQ[i * base_rows + r, gc2:gc2 + n2] = wp[WCOMBO[i]]
                    for i in range(6):
                        if WCOMBO[i] == 0:
                            FQ[i * base_rows + cr, gc2 + n2:gc2 + 128] = BF16(
                                -3.0 * MPEN)
                    for ax in axes:
                        sa = int(span[ax])
                        blo = np.maximum(
                            means_int[gsel[seg], ax] - radii[gsel[seg]] - lo[ax], 0)
                        bhi = np.minimum(
                            means_int[gsel[seg], ax] + radii[gsel[seg]] - lo[ax],
                            sa - 1)
                        k = np.arange(sa)[:, None]
                        box = ((k >= blo[None, :]) & (k <= bhi[None, :]))
                        flat = offs[ax] + np.arange(sa)
                        OH[flat % S2, flat // S2, gc2:gc2 + n2] = np.where(
                            box, FP8_NP(1.0), FP8_NP(0.0))

        ob = np.zeros((128, OBB), np.uint8)
        ob[:, :NG * n_c * 2] = OPA.reshape(128, NG * n_c).view(np.uint8)
        ob[:, NG * n_c * 2 + 4:NG * n_c * 2 + 8] = (
            np.full((128, 1), 1.0, np.float32).view(np.uint8))

        # pack the byte bundle: [fqW | ohW (per-tile) | per-shard fq+oh]
        SR = max(R, S2)
        sh_off = []
        acc = 2 * NG * 256
        for ps in PATTERN:
            sh_off.append(acc)
            acc += 4 * ps
        BUN = np.zeros((SR, acc), np.uint8)
        BUN[:R, 0:NG * 256] = np.ascontiguousarray(
            FQ[:, 0:128 * NG]).view(np.uint8)
        ohw = np.ascontiguousarray(
            OH[:, :, 0:128 * NG].reshape(S2, 2, NG, 128).transpose(0, 2, 1, 3))
        BUN[:S2, NG * 256:2 * NG * 256] = ohw.reshape(S2, NG * 256).view(
            np.uint8)
        for s in range(NS):
            ps = PATTERN[s]
            o = sh_off[s]
            BUN[:R, o:o + 2 * ps] = np.ascontiguousarray(
                FQ[:, CFW + poff[s]:CFW + poff[s] + ps]).view(np.uint8)
            BUN[:S2, o + 2 * ps:o + 4 * ps] = np.ascontiguousarray(
                OH[:, :, COW + poff[s]:COW + poff[s] + ps]).reshape(
                    S2, 2 * ps).view(np.uint8)
        in_maps.append({"bun": BUN, "ob": ob})

    perm = np.concatenate([cores[ci] for ci in range(N_CORES)])
    cfg = (R, S2, PATTERN, gts, n_c)
    return in_maps, perm, cfg


def _run(inputs, trace=False, **run_kwargs):
    in_maps, perm, cfg = _prepare(inputs)
    if cfg not in _nc_cache:
        _nc_cache[cfg] = _build_bass(*cfg)
    nc = _nc_cache[cfg]
    try:
        res = run_bass_kernel_spmd(
            nc, in_maps, core_ids=list(range(N_CORES)), trace=trace, **run_kwargs
        )
    except ModuleNotFoundError:
        res = run_bass_kernel_spmd(
            nc, in_maps, core_ids=list(range(N_CORES)), trace=False, **run_kwargs
        )
    P = P_CORE * N_CORES
    n_c = cfg[4]
    out = np.empty((P, n_c), np.float32)
    for ci in range(N_CORES):
        o = res.results[ci]["out"]  # [128, NB, C]
        out[perm[ci * P_CORE:(ci + 1) * P_CORE]] = (
            o.transpose(1, 0, 2).reshape(P_CORE, n_c))
    return out, res


def kernel(**inputs):
    return _run(inputs)[0]
